# revision 10
# baseline (speedup 1.0000x reference)
"""Residual VQ (3-layer, vq_codebook) Trainium2 kernel.

Data-parallel over 8 NeuronCores: batch rows sharded 1024/core, codebooks
replicated.  Device computes, per core and per layer: t=||r||^2 (DVE
sequential reduce, bitwise-matching the jax-on-neuron eager reference),
u = fl(t + b), mm = (2r)@cbT via native-fp32 PE matmul (bitwise-matching the
eager dot), d = fl(u - mm) (logits), first-index argmin (reduce-min +
max_index, ascending tie order), one-hot via exact Relu(1-(iota-idx)^2) on
the scalar engine, batched codebook row gather, straight-through residual
update and per-row loss partials.  Host does input layout prep (codebook
transpose, ||cb||^2 sums with the same sequential-f32 order the hardware
reduce uses), sharding, and output concatenation + scalar loss finalization.
"""
import numpy as np
from contextlib import ExitStack

import concourse.bass as bass
from concourse import bacc
import concourse.mybir as mybir
from concourse.tile import TileContext
from concourse.bass_utils import run_bass_kernel_spmd
from concourse.masks import make_identity

B = 8192
E_DIM = 256
N_E = 4096
N_LAYERS = 3
BETA = 0.25
N_CORES = 8
BL = B // N_CORES          # rows per core
NT = BL // 128             # row tiles per core
F32 = mybir.dt.float32

_cached = {}


def _build_nc():
    nc = bacc.Bacc()
    xin = nc.dram_tensor("xin", [BL, E_DIM], F32, kind="ExternalInput")
    cbn = [nc.dram_tensor(f"cbn{l}", [N_E, E_DIM], F32, kind="ExternalInput")
           for l in range(N_LAYERS)]
    cbT = [nc.dram_tensor(f"cbT{l}", [E_DIM, N_E], F32, kind="ExternalInput")
           for l in range(N_LAYERS)]
    brep_d = nc.dram_tensor("brep_d", [N_LAYERS, 128, N_E], F32, kind="ExternalInput")
    iota_d = nc.dram_tensor("iota_d", [128, N_E], mybir.dt.int16, kind="ExternalInput")

    logits_o = nc.dram_tensor("logits_o", [BL, N_LAYERS, N_E], F32, kind="ExternalOutput")
    oh_o = nc.dram_tensor("oh_o", [BL, N_LAYERS, N_E], F32, kind="ExternalOutput")
    xq_o = nc.dram_tensor("xq_o", [BL, E_DIM], F32, kind="ExternalOutput")
    idx_o = nc.dram_tensor("idx_o", [BL, N_LAYERS], mybir.dt.int32, kind="ExternalOutput")
    loss_o = nc.dram_tensor("loss_o", [128, N_LAYERS], F32, kind="ExternalOutput")

    AF = mybir.ActivationFunctionType

    with TileContext(nc) as tc, ExitStack() as ctx:
        statics = ctx.enter_context(tc.tile_pool(name="statics", bufs=1))
        cbtp = ctx.enter_context(tc.tile_pool(name="cbtp", bufs=2))
        brepp = ctx.enter_context(tc.tile_pool(name="brepp", bufs=1))
        dpool = ctx.enter_context(tc.tile_pool(name="dpool", bufs=3))
        ohp = ctx.enter_context(tc.tile_pool(name="ohp", bufs=2))
        small = ctx.enter_context(tc.tile_pool(name="small", bufs=2))
        ps_tp = ctx.enter_context(tc.tile_pool(name="ps_tp", bufs=2, space="PSUM"))
        ps_mm = ctx.enter_context(tc.tile_pool(name="ps_mm", bufs=3, space="PSUM"))

        iotaP = statics.tile([128, N_E], mybir.dt.int16)
        nc.gpsimd.dma_start(out=iotaP, in_=iota_d[:, :])
        ident2 = statics.tile([128, 128], F32)
        nc.gpsimd.memset(ident2, 0.0)
        nc.gpsimd.affine_select(
            out=ident2, in_=ident2, compare_op=mybir.AluOpType.not_equal,
            fill=2.0, base=0, pattern=[[-1, 128]], channel_multiplier=1)
        lacc = statics.tile([128, N_LAYERS], F32)

        rts = []
        xqs = []
        for rt in range(NT):
            r_t = statics.tile([128, E_DIM], F32, name=f"r{rt}")
            nc.sync.dma_start(out=r_t, in_=xin[rt * 128:(rt + 1) * 128, :])
            xq_t = statics.tile([128, E_DIM], F32, name=f"xq{rt}")
            rts.append(r_t)
            xqs.append(xq_t)

        for l in range(N_LAYERS):
            cbt0a = cbtp.tile([128, N_E // 2], F32, tag="cbt0a")
            nc.sync.dma_start(out=cbt0a, in_=cbT[l][0:128, 0:N_E // 2])
            cbt1a = cbtp.tile([128, N_E // 2], F32, tag="cbt1a")
            nc.sync.dma_start(out=cbt1a, in_=cbT[l][128:256, 0:N_E // 2])
            cbt0b = cbtp.tile([128, N_E // 2], F32, tag="cbt0b")
            nc.sync.dma_start(out=cbt0b, in_=cbT[l][0:128, N_E // 2:])
            cbt1b = cbtp.tile([128, N_E // 2], F32, tag="cbt1b")
            nc.sync.dma_start(out=cbt1b, in_=cbT[l][128:256, N_E // 2:])
            brep = brepp.tile([128, N_E], F32, tag="brep")
            nc.gpsimd.dma_start(out=brep, in_=brep_d[l, :, :])
            cbthalf = [[cbt0a, cbt0b], [cbt1a, cbt1b]]

            # hoist t = seq-sum(r*r) for all row tiles to layer start
            tts = brepp.tile([128, NT], F32, tag="tts")
            for rt in range(NT):
                sq = small.tile([128, E_DIM], F32, tag="sq")
                nc.vector.tensor_mul(sq, rts[rt], rts[rt])
                nc.vector.reduce_sum(tts[:, rt:rt + 1], sq,
                                     axis=mybir.AxisListType.X)

            for rt in range(NT):
                r_t = rts[rt]
                rows = slice(rt * 128, (rt + 1) * 128)

                rT0 = small.tile([128, 128], F32, tag="rT0")
                rT1 = small.tile([128, 128], F32, tag="rT1")
                rT = [rT0, rT1]
                for k in range(2):
                    tp = ps_tp.tile([128, 128], F32, tag="tp")
                    nc.tensor.matmul(tp, lhsT=r_t[:, k * 128:(k + 1) * 128],
                                     rhs=ident2, start=True, stop=True)
                    nc.vector.tensor_copy(rT[k], tp)

                d_sb = dpool.tile([128, N_E], F32, tag="d")
                nc.scalar.add(d_sb, brep, add=tts[:, rt:rt + 1])

                for jh in range(4):
                    half = jh // 2
                    pm = ps_mm.tile([128, 1024], F32, tag="pm")
                    for s in range(2):
                        cols = slice(((jh % 2) * 2 + s) * 512,
                                     ((jh % 2) * 2 + s + 1) * 512)
                        nc.tensor.matmul(pm[:, s * 512:(s + 1) * 512],
                                         lhsT=rT[0], rhs=cbthalf[0][half][:, cols],
                                         start=True, stop=False)
                        nc.tensor.matmul(pm[:, s * 512:(s + 1) * 512],
                                         lhsT=rT[1], rhs=cbthalf[1][half][:, cols],
                                         start=False, stop=True)
                    hcols = slice(jh * 1024, (jh + 1) * 1024)
                    nc.vector.tensor_sub(d_sb[:, hcols], d_sb[:, hcols], pm)
                nc.sync.dma_start(out=logits_o[rows, l, :], in_=d_sb)

                rowmin = small.tile([128, 1], F32, tag="rowmin")
                nc.vector.tensor_reduce(rowmin, d_sb, axis=mybir.AxisListType.X,
                                        op=mybir.AluOpType.min)
                rm8 = small.tile([128, 8], F32, tag="rm8")
                nc.vector.tensor_copy(rm8, rowmin[:, :1].to_broadcast([128, 8]))
                idx8 = small.tile([128, 8], mybir.dt.uint32, tag="idx8")
                nc.vector.max_index(idx8, rm8, d_sb)
                nc.sync.dma_start(out=idx_o[rows, l:l + 1],
                                  in_=idx8[:, :1].bitcast(mybir.dt.int32))

                # one-hot on DVE: (iota == idx)
                idxf = small.tile([128, 1], F32, tag="idxf")
                nc.scalar.mul(idxf, idx8[:, :1].bitcast(mybir.dt.int32), 1.0)
                ohx = ohp.tile([128, N_E], F32, tag="ohx")
                nc.vector.tensor_scalar(ohx, iotaP, idxf[:, :1], None,
                                        op0=mybir.AluOpType.is_equal)
                nc.sync.dma_start(out=oh_o[rows, l, :], in_=ohx)

                # per-row-tile gather (multi-offset indirect DMA is broken on HW)
                q = small.tile([128, E_DIM], F32, tag="q")
                nc.gpsimd.indirect_dma_start(
                    out=q, out_offset=None, in_=cbn[l][:, :],
                    in_offset=bass.IndirectOffsetOnAxis(
                        ap=idx8[:, :1].bitcast(mybir.dt.int32), axis=0))

                e = small.tile([128, E_DIM], F32, tag="e")
                nc.vector.tensor_sub(e, q, r_t)
                xqst = small.tile([128, E_DIM], F32, tag="xqst")
                nc.vector.tensor_add(xqst, r_t, e)
                if l == 0:
                    nc.vector.tensor_copy(xqs[rt], xqst)
                else:
                    nc.vector.tensor_add(xqs[rt], xqs[rt], xqst)
                nc.vector.tensor_sub(r_t, r_t, xqst)

                e2 = small.tile([128, E_DIM], F32, tag="e2")
                nc.vector.tensor_mul(e2, e, e)
                lsum = small.tile([128, 1], F32, tag="lsum")
                nc.vector.reduce_sum(lsum, e2, axis=mybir.AxisListType.X)
                if rt == 0:
                    nc.vector.tensor_copy(lacc[:, l:l + 1], lsum)
                else:
                    nc.vector.tensor_add(lacc[:, l:l + 1], lacc[:, l:l + 1], lsum)

                if l == N_LAYERS - 1:
                    nc.sync.dma_start(out=xq_o[rows, :], in_=xqs[rt])

        nc.sync.dma_start(out=loss_o[:, :], in_=lacc)

    nc.compile()
    return nc


def _seq_sq_sum(a):
    """Sequential f32 sum of squares over axis 1 (matches the DVE reduce)."""
    a = a.astype(np.float32)
    sq = a * a
    acc = sq[:, 0].copy()
    for k in range(1, a.shape[1]):
        acc = (acc + sq[:, k]).astype(np.float32)
    return acc


def kernel(x, codebooks, trace=False):
    x = np.ascontiguousarray(np.asarray(x, dtype=np.float32))
    cbs = np.ascontiguousarray(np.asarray(codebooks, dtype=np.float32))
    assert x.shape == (B, E_DIM) and cbs.shape == (N_LAYERS, N_E, E_DIM)

    if 'nc' not in _cached:
        _cached['nc'] = _build_nc()
    nc = _cached['nc']

    brep = np.empty((N_LAYERS, 128, N_E), np.float32)
    for l in range(N_LAYERS):
        brep[l] = np.broadcast_to(_seq_sq_sum(cbs[l])[None, :], (128, N_E))
    iota = np.ascontiguousarray(np.broadcast_to(
        np.arange(N_E, dtype=np.int16)[None, :], (128, N_E)))
    cbT = [np.ascontiguousarray(cbs[l].T) for l in range(N_LAYERS)]

    in_maps = []
    for c in range(N_CORES):
        m = {"xin": x[c * BL:(c + 1) * BL],
             "brep_d": brep, "iota_d": iota}
        for l in range(N_LAYERS):
            m[f"cbn{l}"] = cbs[l]
            m[f"cbT{l}"] = cbT[l]
        in_maps.append(m)

    kw = {}
    if trace:
        kw = dict(trace=True)
    out = run_bass_kernel_spmd(nc, in_maps, core_ids=list(range(N_CORES)), **kw)
    res = out.results

    xq = np.concatenate([np.asarray(r["xq_o"]) for r in res], axis=0)
    idx = np.concatenate([np.asarray(r["idx_o"]) for r in res], axis=0).astype(np.int32)
    oh = np.concatenate([np.asarray(r["oh_o"]) for r in res], axis=0)
    logits = np.concatenate([np.asarray(r["logits_o"]) for r in res], axis=0)

    loss_sums = np.zeros(N_LAYERS, np.float64)
    for r in res:
        loss_sums += np.asarray(r["loss_o"]).astype(np.float64).sum(axis=0)
    per_layer = (1.0 + BETA) * (loss_sums / (B * E_DIM))
    mean_loss = np.float32(per_layer.mean())

    if trace:
        kernel.last_exec_time_ns = out.exec_time_ns
    return xq, mean_loss, idx, oh, logits


# revision 11
# speedup vs baseline: 1.1032x; 1.1032x over previous
"""Residual VQ (3-layer, vq_codebook) Trainium2 kernel.

Data-parallel over 8 NeuronCores: batch rows sharded 1024/core, codebooks
replicated.  Device computes, per core and per layer: t=||r||^2 (DVE
sequential reduce, bitwise-matching the jax-on-neuron eager reference),
u = fl(t + b), mm = (2r)@cbT via native-fp32 PE matmul (bitwise-matching the
eager dot), d = fl(u - mm) (logits), first-index argmin (reduce-min +
max_index, ascending tie order), one-hot via exact Relu(1-(iota-idx)^2) on
the scalar engine, batched codebook row gather, straight-through residual
update and per-row loss partials.  Host does input layout prep (codebook
transpose, ||cb||^2 sums with the same sequential-f32 order the hardware
reduce uses), sharding, and output concatenation + scalar loss finalization.
"""
import numpy as np
from contextlib import ExitStack

import concourse.bass as bass
from concourse import bacc
import concourse.mybir as mybir
from concourse.tile import TileContext
from concourse.bass_utils import run_bass_kernel_spmd
from concourse.masks import make_identity

B = 8192
E_DIM = 256
N_E = 4096
N_LAYERS = 3
BETA = 0.25
N_CORES = 8
BL = B // N_CORES          # rows per core
NT = BL // 128             # row tiles per core
F32 = mybir.dt.float32

_cached = {}


def _build_nc():
    nc = bacc.Bacc()
    xin = nc.dram_tensor("xin", [BL, E_DIM], F32, kind="ExternalInput")
    cbn = [nc.dram_tensor(f"cbn{l}", [N_E, E_DIM], F32, kind="ExternalInput")
           for l in range(N_LAYERS)]
    cbT = [nc.dram_tensor(f"cbT{l}", [E_DIM, N_E], F32, kind="ExternalInput")
           for l in range(N_LAYERS)]
    brep_d = nc.dram_tensor("brep_d", [N_LAYERS, 128, N_E], F32, kind="ExternalInput")
    iota_d = nc.dram_tensor("iota_d", [128, N_E], mybir.dt.int16, kind="ExternalInput")

    logits_o = nc.dram_tensor("logits_o", [BL, N_LAYERS, N_E], F32, kind="ExternalOutput")
    oh_o = nc.dram_tensor("oh_o", [BL, N_LAYERS, N_E], F32, kind="ExternalOutput")
    xq_o = nc.dram_tensor("xq_o", [BL, E_DIM], F32, kind="ExternalOutput")
    idx_o = nc.dram_tensor("idx_o", [BL, N_LAYERS], mybir.dt.int32, kind="ExternalOutput")
    loss_o = nc.dram_tensor("loss_o", [128, N_LAYERS], F32, kind="ExternalOutput")

    AF = mybir.ActivationFunctionType

    with TileContext(nc) as tc, ExitStack() as ctx:
        statics = ctx.enter_context(tc.tile_pool(name="statics", bufs=1))
        cbtp = ctx.enter_context(tc.tile_pool(name="cbtp", bufs=2))
        brepp = ctx.enter_context(tc.tile_pool(name="brepp", bufs=2))
        dpool = ctx.enter_context(tc.tile_pool(name="dpool", bufs=2))
        ohp = ctx.enter_context(tc.tile_pool(name="ohp", bufs=2))
        small = ctx.enter_context(tc.tile_pool(name="small", bufs=2))
        ps_tp = ctx.enter_context(tc.tile_pool(name="ps_tp", bufs=2, space="PSUM"))
        ps_mm = ctx.enter_context(tc.tile_pool(name="ps_mm", bufs=3, space="PSUM"))

        iotaP = statics.tile([128, N_E], mybir.dt.int16)
        nc.gpsimd.dma_start(out=iotaP, in_=iota_d[:, :])
        ident2 = statics.tile([128, 128], F32)
        nc.gpsimd.memset(ident2, 0.0)
        nc.gpsimd.affine_select(
            out=ident2, in_=ident2, compare_op=mybir.AluOpType.not_equal,
            fill=2.0, base=0, pattern=[[-1, 128]], channel_multiplier=1)
        lacc = statics.tile([128, N_LAYERS], F32)

        rts = []
        xqs = []
        for rt in range(NT):
            r_t = statics.tile([128, E_DIM], F32, name=f"r{rt}")
            nc.sync.dma_start(out=r_t, in_=xin[rt * 128:(rt + 1) * 128, :])
            xq_t = statics.tile([128, E_DIM], F32, name=f"xq{rt}")
            rts.append(r_t)
            xqs.append(xq_t)

        for l in range(N_LAYERS):
            cbt0a = cbtp.tile([128, N_E // 2], F32, tag="cbt0a")
            nc.sync.dma_start(out=cbt0a, in_=cbT[l][0:128, 0:N_E // 2])
            cbt1a = cbtp.tile([128, N_E // 2], F32, tag="cbt1a")
            nc.sync.dma_start(out=cbt1a, in_=cbT[l][128:256, 0:N_E // 2])
            cbt0b = cbtp.tile([128, N_E // 2], F32, tag="cbt0b")
            nc.sync.dma_start(out=cbt0b, in_=cbT[l][0:128, N_E // 2:])
            cbt1b = cbtp.tile([128, N_E // 2], F32, tag="cbt1b")
            nc.sync.dma_start(out=cbt1b, in_=cbT[l][128:256, N_E // 2:])
            brep = brepp.tile([128, N_E], F32, tag="brep")
            nc.gpsimd.dma_start(out=brep, in_=brep_d[l, :, :])
            cbthalf = [[cbt0a, cbt0b], [cbt1a, cbt1b]]

            for rt in range(NT):
                r_t = rts[rt]
                rows = slice(rt * 128, (rt + 1) * 128)

                sq = small.tile([128, E_DIM], F32, tag="sq")
                nc.vector.tensor_mul(sq, r_t, r_t)
                tt = small.tile([128, 1], F32, tag="tt")
                nc.vector.reduce_sum(tt, sq, axis=mybir.AxisListType.X)

                rT0 = small.tile([128, 128], F32, tag="rT0")
                rT1 = small.tile([128, 128], F32, tag="rT1")
                rT = [rT0, rT1]
                for k in range(2):
                    tp = ps_tp.tile([128, 128], F32, tag="tp")
                    nc.tensor.matmul(tp, lhsT=r_t[:, k * 128:(k + 1) * 128],
                                     rhs=ident2, start=True, stop=True)
                    nc.scalar.copy(rT[k], tp)

                d_sb = dpool.tile([128, N_E], F32, tag="d")
                nc.scalar.add(d_sb, brep, add=tt[:, :1])

                for jh in range(4):
                    half = jh // 2
                    pm = ps_mm.tile([128, 1024], F32, tag="pm")
                    for s in range(2):
                        cols = slice(((jh % 2) * 2 + s) * 512,
                                     ((jh % 2) * 2 + s + 1) * 512)
                        nc.tensor.matmul(pm[:, s * 512:(s + 1) * 512],
                                         lhsT=rT[0], rhs=cbthalf[0][half][:, cols],
                                         start=True, stop=False)
                        nc.tensor.matmul(pm[:, s * 512:(s + 1) * 512],
                                         lhsT=rT[1], rhs=cbthalf[1][half][:, cols],
                                         start=False, stop=True)
                    hcols = slice(jh * 1024, (jh + 1) * 1024)
                    nc.vector.tensor_sub(d_sb[:, hcols], d_sb[:, hcols], pm)
                nc.sync.dma_start(out=logits_o[rows, l, :], in_=d_sb)

                rowmin = small.tile([128, 1], F32, tag="rowmin")
                nc.vector.tensor_reduce(rowmin, d_sb, axis=mybir.AxisListType.X,
                                        op=mybir.AluOpType.min)
                rm8 = small.tile([128, 8], F32, tag="rm8")
                nc.vector.tensor_copy(rm8, rowmin[:, :1].to_broadcast([128, 8]))
                idx8 = small.tile([128, 8], mybir.dt.uint32, tag="idx8")
                nc.vector.max_index(idx8, rm8, d_sb)
                nc.sync.dma_start(out=idx_o[rows, l:l + 1],
                                  in_=idx8[:, :1].bitcast(mybir.dt.int32))

                # one-hot on DVE: (iota == idx)
                idxf = small.tile([128, 1], F32, tag="idxf")
                nc.scalar.mul(idxf, idx8[:, :1].bitcast(mybir.dt.int32), 1.0)
                ohx = ohp.tile([128, N_E], F32, tag="ohx")
                nc.vector.tensor_scalar(ohx, iotaP, idxf[:, :1], None,
                                        op0=mybir.AluOpType.is_equal)
                nc.sync.dma_start(out=oh_o[rows, l, :], in_=ohx)

                # per-row-tile gather (multi-offset indirect DMA is broken on HW)
                q = small.tile([128, E_DIM], F32, tag="q")
                nc.gpsimd.indirect_dma_start(
                    out=q, out_offset=None, in_=cbn[l][:, :],
                    in_offset=bass.IndirectOffsetOnAxis(
                        ap=idx8[:, :1].bitcast(mybir.dt.int32), axis=0))

                e = small.tile([128, E_DIM], F32, tag="e")
                nc.vector.tensor_sub(e, q, r_t)
                xqst = small.tile([128, E_DIM], F32, tag="xqst")
                nc.vector.tensor_add(xqst, r_t, e)
                if l == 0:
                    nc.vector.tensor_copy(xqs[rt], xqst)
                else:
                    nc.vector.tensor_add(xqs[rt], xqs[rt], xqst)
                nc.vector.tensor_sub(r_t, r_t, xqst)

                e2 = small.tile([128, E_DIM], F32, tag="e2")
                nc.vector.tensor_mul(e2, e, e)
                lsum = small.tile([128, 1], F32, tag="lsum")
                nc.vector.reduce_sum(lsum, e2, axis=mybir.AxisListType.X)
                if rt == 0:
                    nc.vector.tensor_copy(lacc[:, l:l + 1], lsum)
                else:
                    nc.vector.tensor_add(lacc[:, l:l + 1], lacc[:, l:l + 1], lsum)

                if l == N_LAYERS - 1:
                    nc.sync.dma_start(out=xq_o[rows, :], in_=xqs[rt])

        nc.sync.dma_start(out=loss_o[:, :], in_=lacc)

    nc.compile()
    return nc


def _seq_sq_sum(a):
    """Sequential f32 sum of squares over axis 1 (matches the DVE reduce)."""
    a = a.astype(np.float32)
    sq = a * a
    acc = sq[:, 0].copy()
    for k in range(1, a.shape[1]):
        acc = (acc + sq[:, k]).astype(np.float32)
    return acc


def kernel(x, codebooks, trace=False):
    x = np.ascontiguousarray(np.asarray(x, dtype=np.float32))
    cbs = np.ascontiguousarray(np.asarray(codebooks, dtype=np.float32))
    assert x.shape == (B, E_DIM) and cbs.shape == (N_LAYERS, N_E, E_DIM)

    if 'nc' not in _cached:
        _cached['nc'] = _build_nc()
    nc = _cached['nc']

    brep = np.empty((N_LAYERS, 128, N_E), np.float32)
    for l in range(N_LAYERS):
        brep[l] = np.broadcast_to(_seq_sq_sum(cbs[l])[None, :], (128, N_E))
    iota = np.ascontiguousarray(np.broadcast_to(
        np.arange(N_E, dtype=np.int16)[None, :], (128, N_E)))
    cbT = [np.ascontiguousarray(cbs[l].T) for l in range(N_LAYERS)]

    in_maps = []
    for c in range(N_CORES):
        m = {"xin": x[c * BL:(c + 1) * BL],
             "brep_d": brep, "iota_d": iota}
        for l in range(N_LAYERS):
            m[f"cbn{l}"] = cbs[l]
            m[f"cbT{l}"] = cbT[l]
        in_maps.append(m)

    kw = {}
    if trace:
        kw = dict(trace=True)
    out = run_bass_kernel_spmd(nc, in_maps, core_ids=list(range(N_CORES)), **kw)
    res = out.results

    xq = np.concatenate([np.asarray(r["xq_o"]) for r in res], axis=0)
    idx = np.concatenate([np.asarray(r["idx_o"]) for r in res], axis=0).astype(np.int32)
    oh = np.concatenate([np.asarray(r["oh_o"]) for r in res], axis=0)
    logits = np.concatenate([np.asarray(r["logits_o"]) for r in res], axis=0)

    loss_sums = np.zeros(N_LAYERS, np.float64)
    for r in res:
        loss_sums += np.asarray(r["loss_o"]).astype(np.float64).sum(axis=0)
    per_layer = (1.0 + BETA) * (loss_sums / (B * E_DIM))
    mean_loss = np.float32(per_layer.mean())

    if trace:
        kernel.last_exec_time_ns = out.exec_time_ns
    return xq, mean_loss, idx, oh, logits
